# revision 19
# baseline (speedup 1.0000x reference)
"""Trainium2 kernel for nn_LocalSpectralAdapter.

Math: the reference rfft/irfft only modifies 16 frequency bins, so
  out = x + irfft(sparse delta-spectrum)
which is a rank-32 DFT analysis + rank-64 weighted synthesis:

  P  = F4.T @ x_b            [128, 512]  (Xr/Xi of the 16 bins, laid out twice
                                          in two different row orders)
  TT = P * G12               [128, 512]  (complex gain application, one
                                          elementwise mult; signs folded in)
  y  = I.T @ x_b + Ginv2.T @ TT          (crossfade weights ew/(1-ew) and the
                                          2/T irfft scale folded into Ginv2;
                                          the x residual is accumulated in
                                          PSUM by an identity matmul)

B=64 is sharded 8 ways across cores (pure data parallel, 8 batch/core).

v2: the fp32 version was pinned to the ~358 GB/s per-core HBM roofline
(16 MiB in + 16 MiB out = ~94 us minimum). All device I/O is now bf16
(host casts x down, upcasts y), halving HBM bytes -> ~47 us roofline.
The residual add rides the tensor engine (identity matmul into the same
PSUM accumulation as the synthesis matmul) because a DVE tensor_tensor
add from fp32 PSUM runs at 1x and would itself approach the roofline.
PSUM->SBUF bf16 evacuation alternates between the vector and scalar
engines so neither becomes critical. Loads issue on the sync HWDGE ring,
stores on the scalar ring, both at half-batch (512 KB) granularity.
"""

import numpy as np
import ml_dtypes

_T = 1024
_V = 512
_B = 64
_NCORES = 8
_BPC = _B // _NCORES  # batch per core
_NCHUNK = _T // 128  # 8 t-chunks of 128
_BINS = np.array([1, 2, 3, 4, 5, 6, 7, 8, 12, 16, 24, 32, 48, 64, 96, 128])
_FADE_START = 487
_FADE_END = 537

_BF16 = ml_dtypes.bfloat16


def _static_transforms():
    """F4 [128,1024] (forward lhsT chunks) and Ginv2 [128,1024] (inverse lhsT),
    both independent of the gain inputs. bf16 for 1-row/cycle matmul streaming
    and FWL weight loads."""
    t = np.arange(_T, dtype=np.float64)
    w = 2.0 * np.pi * np.outer(t, _BINS) / _T  # [1024, 16]
    C = np.cos(w)
    S = np.sin(w)

    # Forward: PSUM rows = [Xr, Xi, Xr, Xi | Xi, Xr, Xi, Xr] blocks of 16.
    F4 = np.concatenate([C, -S, C, -S, -S, C, -S, C], axis=1)  # [1024, 128]
    # SBUF partition p holds the contiguous t-range [8p, 8p+8) (so each DMA
    # partition line is one contiguous DRAM run); matmul chunk q uses
    # t = 8p + q, i.e. lhsT chunk q at f4_dram[:, 128q:128(q+1)] with
    # f4_dram[p, 128q + m] = F4[8p + q, m].
    f4_dram = np.ascontiguousarray(F4.reshape(128, _NCHUNK * 128)).astype(_BF16)

    fade = 1.0 - (t - _FADE_START) / (_FADE_END - _FADE_START)
    ew = np.where(t < _FADE_START, 1.0, np.where(t < _FADE_END, fade, 0.0))

    s = 2.0 / _T
    Ginv = np.concatenate(
        [s * ew * C.T, -s * ew * S.T, s * (1.0 - ew) * C.T, -s * (1.0 - ew) * S.T],
        axis=0,
    )  # [64, 1024] channels x t
    Ginv2 = np.concatenate([Ginv, Ginv], axis=0)  # [128ch, 1024t]
    # inverse lhsT chunk q: ginv2_dram[ch, 128q + p] = Ginv2[ch, 8p + q]
    ginv2_dram = np.ascontiguousarray(
        Ginv2.reshape(128, 128, _NCHUNK).transpose(0, 2, 1).reshape(128, _T)
    ).astype(_BF16)
    ident_dram = np.eye(128, dtype=np.float32).astype(_BF16)
    return f4_dram, ginv2_dram, ident_dram


def _gain_matrix(ger, gei, glr, gli):
    """G12 [128,512]: per-channel gain factors aligned with the PSUM row order,
    with the +/- signs of the complex multiply folded in."""
    return np.ascontiguousarray(
        np.concatenate(
            [ger.T, ger.T, glr.T, glr.T, -gei.T, gei.T, -gli.T, gli.T], axis=0
        )
    ).astype(np.float32)


_CACHED_NC = None


def _build_bass():
    global _CACHED_NC
    if _CACHED_NC is not None:
        return _CACHED_NC

    import concourse.mybir as mybir
    from concourse import bacc
    from concourse.tile import TileContext

    f32 = mybir.dt.float32
    bf16 = mybir.dt.bfloat16
    nc = bacc.Bacc("TRN2", target_bir_lowering=False, debug=False)

    x = nc.dram_tensor("x", [_BPC, _T, _V], bf16, kind="ExternalInput").ap()
    f4 = nc.dram_tensor("f4", [128, _NCHUNK * 128], bf16, kind="ExternalInput").ap()
    ginv2 = nc.dram_tensor("ginv2", [128, _T], bf16, kind="ExternalInput").ap()
    ident = nc.dram_tensor("ident", [128, 128], bf16, kind="ExternalInput").ap()
    g12 = nc.dram_tensor("g12", [128, _V], f32, kind="ExternalInput").ap()
    y = nc.dram_tensor("y", [_BPC, _T, _V], bf16, kind="ExternalOutput").ap()

    HB = _NCHUNK * _V // 2  # 2048: half-batch free-dim span

    with TileContext(nc) as tc:
        with (
            tc.tile_pool(name="const", bufs=1) as cpool,
            tc.tile_pool(name="xin", bufs=8) as xpool,
            tc.tile_pool(name="yout", bufs=5) as ypool,
            tc.tile_pool(name="coef", bufs=2) as ttpool,
            tc.tile_pool(name="pfwd", bufs=2, space="PSUM") as ppool,
            tc.tile_pool(name="pinv", bufs=3, space="PSUM") as qpool,
        ):
            # Everything batch 0 needs goes serially at the FRONT of the sync
            # HWDGE queue, ordered by first use, BEFORE the 7 MB of batch
            # prefetch — constants placed on any other queue get starved by
            # the prefetch stream's packet round-robin and then head-of-line
            # block the in-order tensor queue (Tile hoists the identity
            # matmuls ahead of fwd c4-c7, so `ident` must land early).
            # All 8 batch loads are pre-issued (xpool bufs=8, so no slot
            # waits) — the store triggers later in the loop queue up behind
            # them on the sync ring without delaying any load descriptors.
            # Fine-grained first loads: the first forward matmul needs only
            # f4's chunk-0 slice (32 KB) and batch-0's first quarter (256 KB),
            # so it starts ~10.6us and the cold-HAM warmup overlaps the rest
            # of the head loads instead of following them.
            # PE warm-up: the HAM clock gate holds the PE at 1.2 GHz until
            # ~3.4us of sustained matmul activity. Real data is not ready
            # until ~11.5us (transfer + ~2us DMA receipt), so a memset tile
            # (no DMA dependency, runs right after the ~7.2us boot barrier)
            # feeds dummy matmuls that flip the gate before the first real
            # matmul — batch 0 then runs at 2.4 GHz instead of half rate.
            wtile = cpool.tile([128, _V], bf16)
            nc.vector.memset(wtile[:], 0.0)
            wp = ppool.tile([128, _V], f32, tag="P")
            for _ in range(7):
                nc.tensor.matmul(
                    wp[:], lhsT=wtile[:, 0:128], rhs=wtile[:], start=True, stop=True
                )

            f4sb = cpool.tile([128, _NCHUNK * 128], bf16)
            nc.sync.dma_start(out=f4sb[:, 0:128], in_=f4[:, 0:128])

            xsbs = {}
            xsbs[0] = xpool.tile([128, _NCHUNK * _V], bf16, tag="xsb", name="xsb")
            xv0 = x[0].rearrange("(p q) v -> p (q v)", p=128)
            nc.sync.dma_start(out=xsbs[0][:, 0:HB], in_=xv0[:, 0:HB])

            nc.sync.dma_start(out=f4sb[:, 128:], in_=f4[:, 128:])

            identsb = cpool.tile([128, 128], bf16)
            nc.sync.dma_start(out=identsb[:], in_=ident[:])

            nc.sync.dma_start(out=xsbs[0][:, HB:], in_=xv0[:, HB:])

            ginv2sb = cpool.tile([128, _T], bf16)
            nc.sync.dma_start(out=ginv2sb[:], in_=ginv2[:])
            g12sb = cpool.tile([128, _V], f32)
            nc.sync.dma_start(out=g12sb[:], in_=g12[:])

            for b in range(1, _BPC):
                xsbs[b] = xpool.tile([128, _NCHUNK * _V], bf16, tag="xsb", name="xsb")
                xv = x[b].rearrange("(p q) v -> p (q v)", p=128)
                nc.sync.dma_start(out=xsbs[b][:], in_=xv[:])

            for b in range(_BPC):
                xsb = xsbs[b]

                # Forward DFT at the 16 bins, accumulated over the 8 t-chunks.
                P = ppool.tile([128, _V], f32)
                for c in range(_NCHUNK):
                    nc.tensor.matmul(
                        P[:],
                        lhsT=f4sb[:, c * 128 : (c + 1) * 128],
                        rhs=xsb[:, c * _V : (c + 1) * _V],
                        start=(c == 0),
                        stop=(c == _NCHUNK - 1),
                    )

                # Complex gain application: one elementwise multiply; the DVE
                # output stage rounds to bf16 for the synthesis matmul.
                tt = ttpool.tile([128, _V], bf16)
                nc.vector.tensor_mul(tt[:], P[:], g12sb[:])

                # Weighted synthesis. Groups 0-1 carry the x residual on the
                # tensor engine (identity matmul accumulated into the same
                # PSUM bank as the synthesis matmul; fp32 PSUM -> bf16 SBUF
                # evacuation is a plain scalar-engine copy); groups 2-3 are
                # synthesis-only with the residual added by a vector
                # tensor_add. Stores ride the sync HWDGE ring (the SWDGE
                # path moves only ~1 store/1.8us and stretched the kernel
                # tail by ~8us); their triggers just wait on the producer
                # semaphores, so neither compute engine's queue is coupled
                # to DMA completion.
                ysb = ypool.tile([128, _NCHUNK * _V], bf16)
                yv = y[b].rearrange("(p q) v -> p (q v)", p=128)
                for g in range(_NCHUNK // 2):
                    Q = qpool.tile([128, 2 * _V], f32)
                    if g < 2:
                        for h in range(2):
                            c = 2 * g + h
                            nc.tensor.matmul(
                                Q[:, h * _V : (h + 1) * _V],
                                lhsT=identsb[:],
                                rhs=xsb[:, c * _V : (c + 1) * _V],
                                start=True,
                                stop=False,
                            )
                        for h in range(2):
                            c = 2 * g + h
                            nc.tensor.matmul(
                                Q[:, h * _V : (h + 1) * _V],
                                lhsT=ginv2sb[:, c * 128 : (c + 1) * 128],
                                rhs=tt[:],
                                start=False,
                                stop=True,
                            )
                        nc.scalar.copy(ysb[:, 2 * g * _V : (2 * g + 2) * _V], Q[:])
                    else:
                        for h in range(2):
                            c = 2 * g + h
                            nc.tensor.matmul(
                                Q[:, h * _V : (h + 1) * _V],
                                lhsT=ginv2sb[:, c * 128 : (c + 1) * 128],
                                rhs=tt[:],
                                start=True,
                                stop=True,
                            )
                        nc.vector.tensor_add(
                            ysb[:, 2 * g * _V : (2 * g + 2) * _V],
                            Q[:],
                            xsb[:, 2 * g * _V : (2 * g + 2) * _V],
                        )
                    # Tile pairs every dma_start with a completion-lane wait
                    # on the issuing engine's queue, so a queue carrying more
                    # than ~1 store per batch cycle serializes the back half
                    # of the kernel. Batches 0-5 store once (1 MB, scalar
                    # ring); batch 6 in halves and batch 7 in quarters spread
                    # across both HWDGE rings, so the end-of-kernel store
                    # backlog is one 256 KB transfer instead of ~2 MB.
                    QB = 2 * _V  # one group = quarter batch
                    if b == _BPC - 1:
                        eng = nc.scalar if g < 2 else nc.sync
                        eng.dma_start(
                            out=yv[:, g * QB : (g + 1) * QB],
                            in_=ysb[:, g * QB : (g + 1) * QB],
                        )
                    elif b == _BPC - 2:
                        if g == 1:
                            nc.scalar.dma_start(
                                out=yv[:, 0:HB], in_=ysb[:, 0:HB]
                            )
                        elif g == 3:
                            nc.sync.dma_start(
                                out=yv[:, HB:], in_=ysb[:, HB:]
                            )
                    elif g == 3:
                        nc.scalar.dma_start(out=yv[:], in_=ysb[:])

    nc.compile()
    _CACHED_NC = nc
    return nc


def _run(x, g_early_real, g_early_imag, g_late_real, g_late_imag, **spmd_kwargs):
    """Shard inputs, run the Bass kernel on 8 cores, return BassKernelResults."""
    from concourse.bass_utils import run_bass_kernel_spmd

    g_early_real = np.asarray(g_early_real, dtype=np.float32)
    g_early_imag = np.asarray(g_early_imag, dtype=np.float32)
    g_late_real = np.asarray(g_late_real, dtype=np.float32)
    g_late_imag = np.asarray(g_late_imag, dtype=np.float32)
    f4_dram, ginv2_dram, ident_dram = _static_transforms()
    g12_dram = _gain_matrix(g_early_real, g_early_imag, g_late_real, g_late_imag)

    xb = np.asarray(x).astype(_BF16)  # round-to-nearest-even cast, host side
    nc = _build_bass()

    in_maps = [
        {
            "x": xb[i * _BPC : (i + 1) * _BPC],
            "f4": f4_dram,
            "ginv2": ginv2_dram,
            "ident": ident_dram,
            "g12": g12_dram,
        }
        for i in range(_NCORES)
    ]
    return run_bass_kernel_spmd(
        nc, in_maps, core_ids=list(range(_NCORES)), **spmd_kwargs
    )


def kernel(x, g_early_real, g_early_imag, g_late_real, g_late_imag):
    import time

    last = None
    for _attempt in range(3):
        try:
            res = _run(x, g_early_real, g_early_imag, g_late_real, g_late_imag)
            return np.concatenate(
                [np.asarray(r["y"], dtype=np.float32) for r in res.results], axis=0
            )
        except Exception as e:
            # The axon-tunneled NeuronCores occasionally report a transient
            # NRT_EXEC_UNIT_UNRECOVERABLE right after a prior heavy run;
            # a short backoff and retry clears it.
            last = e
            msg = str(e)
            if "UNRECOVER" in msg or "UNAVAILABLE" in msg:
                time.sleep(5.0)
                continue
            raise
    raise last


# revision 21
# speedup vs baseline: 1.1034x; 1.1034x over previous
"""Trainium2 kernel for nn_LocalSpectralAdapter.

Math: the reference rfft/irfft only modifies 16 frequency bins, so
  out = x + irfft(sparse delta-spectrum)
which is a rank-32 DFT analysis + rank-64 weighted synthesis:

  P  = F4.T @ x_b            [128, 512]  (Xr/Xi of the 16 bins, laid out twice
                                          in two different row orders)
  TT = P * G12               [128, 512]  (complex gain application, one
                                          elementwise mult; signs folded in)
  y  = I.T @ x_b + Ginv2.T @ TT          (crossfade weights ew/(1-ew) and the
                                          2/T irfft scale folded into Ginv2;
                                          the x residual is accumulated in
                                          PSUM by an identity matmul)

B=64 is sharded 8 ways across cores (pure data parallel, 8 batch/core).

v2: the fp32 version was pinned to the ~358 GB/s per-core HBM roofline
(16 MiB in + 16 MiB out = ~94 us minimum). All device I/O is now bf16
(host casts x down, upcasts y), halving HBM bytes -> ~47 us roofline.
The residual add rides the tensor engine (identity matmul into the same
PSUM accumulation as the synthesis matmul) because a DVE tensor_tensor
add from fp32 PSUM runs at 1x and would itself approach the roofline.
PSUM->SBUF bf16 evacuation alternates between the vector and scalar
engines so neither becomes critical. Loads issue on the sync HWDGE ring,
stores on the scalar ring, both at half-batch (512 KB) granularity.
"""

import numpy as np
import ml_dtypes

_T = 1024
_V = 512
_B = 64
_NCORES = 8
_BPC = _B // _NCORES  # batch per core
_NCHUNK = _T // 128  # 8 t-chunks of 128
_BINS = np.array([1, 2, 3, 4, 5, 6, 7, 8, 12, 16, 24, 32, 48, 64, 96, 128])
_FADE_START = 487
_FADE_END = 537

_BF16 = ml_dtypes.bfloat16


def _static_transforms():
    """F4 [128,1024] (forward lhsT chunks) and Ginv2 [128,1024] (inverse lhsT),
    both independent of the gain inputs. bf16 for 1-row/cycle matmul streaming
    and FWL weight loads."""
    t = np.arange(_T, dtype=np.float64)
    w = 2.0 * np.pi * np.outer(t, _BINS) / _T  # [1024, 16]
    C = np.cos(w)
    S = np.sin(w)

    # Forward: PSUM rows = [Xr, Xi, Xr, Xi | Xi, Xr, Xi, Xr] blocks of 16.
    F4 = np.concatenate([C, -S, C, -S, -S, C, -S, C], axis=1)  # [1024, 128]
    # SBUF partition p holds the contiguous t-range [8p, 8p+8) (so each DMA
    # partition line is one contiguous DRAM run); matmul chunk q uses
    # t = 8p + q, i.e. lhsT chunk q at f4_dram[:, 128q:128(q+1)] with
    # f4_dram[p, 128q + m] = F4[8p + q, m].
    f4_dram = np.ascontiguousarray(F4.reshape(128, _NCHUNK * 128)).astype(_BF16)

    fade = 1.0 - (t - _FADE_START) / (_FADE_END - _FADE_START)
    ew = np.where(t < _FADE_START, 1.0, np.where(t < _FADE_END, fade, 0.0))

    s = 2.0 / _T
    Ginv = np.concatenate(
        [s * ew * C.T, -s * ew * S.T, s * (1.0 - ew) * C.T, -s * (1.0 - ew) * S.T],
        axis=0,
    )  # [64, 1024] channels x t
    Ginv2 = np.concatenate([Ginv, Ginv], axis=0)  # [128ch, 1024t]
    # inverse lhsT chunk q: ginv2_dram[ch, 128q + p] = Ginv2[ch, 8p + q]
    ginv2_dram = np.ascontiguousarray(
        Ginv2.reshape(128, 128, _NCHUNK).transpose(0, 2, 1).reshape(128, _T)
    ).astype(_BF16)
    ident_dram = np.eye(128, dtype=np.float32).astype(_BF16)
    return f4_dram, ginv2_dram, ident_dram


def _gain_matrix(ger, gei, glr, gli):
    """G12 [128,512]: per-channel gain factors aligned with the PSUM row order,
    with the +/- signs of the complex multiply folded in."""
    return np.ascontiguousarray(
        np.concatenate(
            [ger.T, ger.T, glr.T, glr.T, -gei.T, gei.T, -gli.T, gli.T], axis=0
        )
    ).astype(np.float32)


_CACHED_NC = None


def _build_bass():
    global _CACHED_NC
    if _CACHED_NC is not None:
        return _CACHED_NC

    import concourse.mybir as mybir
    from concourse import bacc
    from concourse.tile import TileContext

    f32 = mybir.dt.float32
    bf16 = mybir.dt.bfloat16
    nc = bacc.Bacc("TRN2", target_bir_lowering=False, debug=False)

    x = nc.dram_tensor("x", [_BPC, _T, _V], bf16, kind="ExternalInput").ap()
    f4 = nc.dram_tensor("f4", [128, _NCHUNK * 128], bf16, kind="ExternalInput").ap()
    ginv2 = nc.dram_tensor("ginv2", [128, _T], bf16, kind="ExternalInput").ap()
    ident = nc.dram_tensor("ident", [128, 128], bf16, kind="ExternalInput").ap()
    g12 = nc.dram_tensor("g12", [128, _V], f32, kind="ExternalInput").ap()
    y = nc.dram_tensor("y", [_BPC, _T, _V], bf16, kind="ExternalOutput").ap()

    HB = _NCHUNK * _V // 2  # 2048: half-batch free-dim span

    with TileContext(nc) as tc:
        with (
            tc.tile_pool(name="const", bufs=1) as cpool,
            tc.tile_pool(name="xin", bufs=8) as xpool,
            tc.tile_pool(name="yout", bufs=5) as ypool,
            tc.tile_pool(name="coef", bufs=2) as ttpool,
            tc.tile_pool(name="pfwd", bufs=2, space="PSUM") as ppool,
            tc.tile_pool(name="pinv", bufs=3, space="PSUM") as qpool,
        ):
            # Everything batch 0 needs goes serially at the FRONT of the sync
            # HWDGE queue, ordered by first use, BEFORE the 7 MB of batch
            # prefetch — constants placed on any other queue get starved by
            # the prefetch stream's packet round-robin and then head-of-line
            # block the in-order tensor queue (Tile hoists the identity
            # matmuls ahead of fwd c4-c7, so `ident` must land early).
            # All 8 batch loads are pre-issued (xpool bufs=8, so no slot
            # waits) — the store triggers later in the loop queue up behind
            # them on the sync ring without delaying any load descriptors.
            # Fine-grained first loads: the first forward matmul needs only
            # f4's chunk-0 slice (32 KB) and batch-0's first quarter (256 KB),
            # so it starts ~10.6us and the cold-HAM warmup overlaps the rest
            # of the head loads instead of following them.
            # PE warm-up: the HAM clock gate holds the PE at 1.2 GHz until
            # ~3.4us of sustained matmul activity. Real data is not ready
            # until ~11.5us (transfer + ~2us DMA receipt), so a memset tile
            # (no DMA dependency, runs right after the ~7.2us boot barrier)
            # feeds dummy matmuls that flip the gate before the first real
            # matmul — batch 0 then runs at 2.4 GHz instead of half rate.
            wtile = cpool.tile([128, _V], bf16)
            nc.vector.memset(wtile[:], 0.0)
            wp = ppool.tile([128, _V], f32, tag="P")
            for _ in range(7):
                nc.tensor.matmul(
                    wp[:], lhsT=wtile[:, 0:128], rhs=wtile[:], start=True, stop=True
                )

            f4sb = cpool.tile([128, _NCHUNK * 128], bf16)
            nc.sync.dma_start(out=f4sb[:, 0:128], in_=f4[:, 0:128])

            xsbs = {}
            xsbs[0] = xpool.tile([128, _NCHUNK * _V], bf16, tag="xsb", name="xsb")
            xv0 = x[0].rearrange("(p q) v -> p (q v)", p=128)
            nc.sync.dma_start(out=xsbs[0][:, 0:HB], in_=xv0[:, 0:HB])

            nc.sync.dma_start(out=f4sb[:, 128:], in_=f4[:, 128:])

            identsb = cpool.tile([128, 128], bf16)
            nc.sync.dma_start(out=identsb[:], in_=ident[:])

            nc.sync.dma_start(out=xsbs[0][:, HB:], in_=xv0[:, HB:])

            ginv2sb = cpool.tile([128, _T], bf16)
            nc.sync.dma_start(out=ginv2sb[:], in_=ginv2[:])
            g12sb = cpool.tile([128, _V], f32)
            nc.sync.dma_start(out=g12sb[:], in_=g12[:])

            for b in range(1, _BPC):
                xsbs[b] = xpool.tile([128, _NCHUNK * _V], bf16, tag="xsb", name="xsb")
                xv = x[b].rearrange("(p q) v -> p (q v)", p=128)
                nc.sync.dma_start(out=xsbs[b][:], in_=xv[:])

            for b in range(_BPC):
                xsb = xsbs[b]

                # Forward DFT at the 16 bins, accumulated over the 8 t-chunks.
                P = ppool.tile([128, _V], f32)
                for c in range(_NCHUNK):
                    nc.tensor.matmul(
                        P[:],
                        lhsT=f4sb[:, c * 128 : (c + 1) * 128],
                        rhs=xsb[:, c * _V : (c + 1) * _V],
                        start=(c == 0),
                        stop=(c == _NCHUNK - 1),
                    )

                # Complex gain application: one elementwise multiply; the DVE
                # output stage rounds to bf16 for the synthesis matmul.
                tt = ttpool.tile([128, _V], bf16)
                nc.vector.tensor_mul(tt[:], P[:], g12sb[:])

                # Weighted synthesis. Groups 0-1 carry the x residual on the
                # tensor engine (identity matmul accumulated into the same
                # PSUM bank as the synthesis matmul; fp32 PSUM -> bf16 SBUF
                # evacuation is a plain scalar-engine copy); groups 2-3 are
                # synthesis-only with the residual added by a vector
                # tensor_add. Stores ride the sync HWDGE ring (the SWDGE
                # path moves only ~1 store/1.8us and stretched the kernel
                # tail by ~8us); their triggers just wait on the producer
                # semaphores, so neither compute engine's queue is coupled
                # to DMA completion.
                ysb = ypool.tile([128, _NCHUNK * _V], bf16)
                yv = y[b].rearrange("(p q) v -> p (q v)", p=128)
                # Last batch: 3 identity groups, so after its final matmul the
                # scalar copy (g2) and the single vector add (g3) run in
                # parallel — ~1us shorter exec tail.
                n_ident = 3 if b == _BPC - 1 else 2
                for g in range(_NCHUNK // 2):
                    Q = qpool.tile([128, 2 * _V], f32)
                    if g < n_ident:
                        for h in range(2):
                            c = 2 * g + h
                            nc.tensor.matmul(
                                Q[:, h * _V : (h + 1) * _V],
                                lhsT=identsb[:],
                                rhs=xsb[:, c * _V : (c + 1) * _V],
                                start=True,
                                stop=False,
                            )
                        for h in range(2):
                            c = 2 * g + h
                            nc.tensor.matmul(
                                Q[:, h * _V : (h + 1) * _V],
                                lhsT=ginv2sb[:, c * 128 : (c + 1) * 128],
                                rhs=tt[:],
                                start=False,
                                stop=True,
                            )
                        nc.scalar.copy(ysb[:, 2 * g * _V : (2 * g + 2) * _V], Q[:])
                    else:
                        for h in range(2):
                            c = 2 * g + h
                            nc.tensor.matmul(
                                Q[:, h * _V : (h + 1) * _V],
                                lhsT=ginv2sb[:, c * 128 : (c + 1) * 128],
                                rhs=tt[:],
                                start=True,
                                stop=True,
                            )
                        nc.vector.tensor_add(
                            ysb[:, 2 * g * _V : (2 * g + 2) * _V],
                            Q[:],
                            xsb[:, 2 * g * _V : (2 * g + 2) * _V],
                        )
                    # Tile pairs every dma_start with a completion-lane wait
                    # on the issuing engine's queue, so a queue carrying more
                    # than ~1 store per batch cycle serializes the back half
                    # of the kernel. Batches 0-5 store once (1 MB, scalar
                    # ring); batch 6 in halves and batch 7 in quarters spread
                    # across both HWDGE rings, so the end-of-kernel store
                    # backlog is one 256 KB transfer instead of ~2 MB.
                    QB = 2 * _V  # one group = quarter batch
                    if b == _BPC - 1:
                        eng = nc.scalar if g < 3 else nc.sync
                        eng.dma_start(
                            out=yv[:, g * QB : (g + 1) * QB],
                            in_=ysb[:, g * QB : (g + 1) * QB],
                        )
                    elif b == _BPC - 2:
                        if g == 1:
                            nc.scalar.dma_start(
                                out=yv[:, 0:HB], in_=ysb[:, 0:HB]
                            )
                        elif g == 3:
                            nc.sync.dma_start(
                                out=yv[:, HB:], in_=ysb[:, HB:]
                            )
                    elif g == 3:
                        nc.scalar.dma_start(out=yv[:], in_=ysb[:])

    nc.compile()
    _CACHED_NC = nc
    return nc


def _run(x, g_early_real, g_early_imag, g_late_real, g_late_imag, **spmd_kwargs):
    """Shard inputs, run the Bass kernel on 8 cores, return BassKernelResults."""
    from concourse.bass_utils import run_bass_kernel_spmd

    g_early_real = np.asarray(g_early_real, dtype=np.float32)
    g_early_imag = np.asarray(g_early_imag, dtype=np.float32)
    g_late_real = np.asarray(g_late_real, dtype=np.float32)
    g_late_imag = np.asarray(g_late_imag, dtype=np.float32)
    f4_dram, ginv2_dram, ident_dram = _static_transforms()
    g12_dram = _gain_matrix(g_early_real, g_early_imag, g_late_real, g_late_imag)

    xb = np.asarray(x).astype(_BF16)  # round-to-nearest-even cast, host side
    nc = _build_bass()

    in_maps = [
        {
            "x": xb[i * _BPC : (i + 1) * _BPC],
            "f4": f4_dram,
            "ginv2": ginv2_dram,
            "ident": ident_dram,
            "g12": g12_dram,
        }
        for i in range(_NCORES)
    ]
    return run_bass_kernel_spmd(
        nc, in_maps, core_ids=list(range(_NCORES)), **spmd_kwargs
    )


def kernel(x, g_early_real, g_early_imag, g_late_real, g_late_imag):
    import time

    last = None
    for _attempt in range(3):
        try:
            res = _run(x, g_early_real, g_early_imag, g_late_real, g_late_imag)
            return np.concatenate(
                [np.asarray(r["y"], dtype=np.float32) for r in res.results], axis=0
            )
        except Exception as e:
            # The axon-tunneled NeuronCores occasionally report a transient
            # NRT_EXEC_UNIT_UNRECOVERABLE right after a prior heavy run;
            # a short backoff and retry clears it.
            last = e
            msg = str(e)
            if "UNRECOVER" in msg or "UNAVAILABLE" in msg:
                time.sleep(5.0)
                continue
            raise
    raise last
